# revision 2
# baseline (speedup 1.0000x reference)
# Trainium2 Bass kernel for NormalAttention (1x1-conv q/k/v attention over HW).
#
# Math (per batch b, one NeuronCore):
#   q = Wq x + bq            [64, 4096]
#   k = Wk x + bk            [64, 4096]
#   v = Wv x + bv            [256, 4096]
#   E[i,j] = sum_c q[c,i] k[c,j]
#   A = elu(E) / 4096
#   out = v @ A              [256, 4096]
#   y = Wg out + bg
#
# Decomposition used here (exact):
#   G  = elu(E) + 1 = t + r + E,  r = relu(-E), t = exp(-r)
#   (v/N) @ G = (v/N) @ (t + r)  +  P @ k,   P = (v/N) @ q^T   (low rank)
#   y = Wg out' + (bg - Wg S),   S = rowsum(v/N)   (corrects the +1)
#
# So the elementwise chain per E-tile is:
#   r  = (eps * -1) max 0     one PSUM->SBUF pass (DVE, some on ACT)
#   t  = exp(-r)              ACT, batched EGRP pairs wide to amortize
#   g' = t + r                DVE (some on Pool), bf16 SBUF 2x mode
# and the E-term costs no elementwise work at all (PE matmuls).
#
# Data-parallel: one batch per core, 8 cores. Software-pipelined with an
# out-matmul lag of LAG_O pairs so PE / ACT / DVE streams overlap instead
# of ping-ponging.
import os

import numpy as np
import ml_dtypes

import concourse.bass as bass
import concourse.mybir as mybir
import concourse.tile as tile
from concourse import bacc
from concourse.bass_utils import run_bass_kernel_spmd

B, C, HH, WW = 8, 256, 64, 64
N = HH * WW          # 4096 spatial positions
CQ = 64              # query/key channels
CVQ = C + CQ         # fused v|q column block
NCORES = 8
MT = 512             # energy column tile
NP = 16              # pairs per m-tile (2x128 rows each)
NPAIR = (N // MT) * NP          # 128
EGRP = int(os.environ.get("K_EGRP", "4"))       # pairs per batched exp
LAG_C = int(os.environ.get("K_LAGC", str(EGRP + 1)))
LAG_O = int(os.environ.get("K_LAGO", str(EGRP + 2)))
ACT_R_EVERY = int(os.environ.get("K_ACTR", "6"))    # k-th r-op on ACT
POOL_COMB_EVERY = int(os.environ.get("K_POOL", "4"))  # k-th combine on Pool
EPS_BUFS = int(os.environ.get("K_EPSB", "2"))
GAMMA_ON_EPS = int(os.environ.get("K_GEPS", "0"))   # gamma psum borrows eps

F32 = mybir.dt.float32
F32R = mybir.dt.float32r
BF16 = mybir.dt.bfloat16
AL = mybir.AluOpType
AF = mybir.ActivationFunctionType


def build_nc(reps=1):
    nc = bacc.Bacc("TRN2", target_bir_lowering=False, debug=False,
                   num_devices=NCORES)
    xd = nc.declare_dram_parameter("x", [2, 128, N], F32R, isOutput=False)
    wqd = nc.declare_dram_parameter("wqT", [2, 128, CQ], F32R, isOutput=False)
    wkd = nc.declare_dram_parameter("wkT", [2, 128, CQ], F32R, isOutput=False)
    bqd = nc.declare_dram_parameter("bq", [CQ, 1], F32, isOutput=False)
    bkd = nc.declare_dram_parameter("bk", [CQ, 1], F32, isOutput=False)
    # fused [wv.T/N | wq.T] and [bv/N | bq] for the v^T / q^T pass
    wvqd = nc.declare_dram_parameter("wvqT", [2, 128, CVQ], F32R,
                                     isOutput=False)
    bvqd = nc.declare_dram_parameter("bvq", [1, CVQ], F32R, isOutput=False)
    wgd = nc.declare_dram_parameter("wgT16", [2, 128, C], BF16, isOutput=False)
    bgd = nc.declare_dram_parameter("bg", [C, 1], F32, isOutput=False)
    onesd = nc.declare_dram_parameter("ones", [1, 128], F32R, isOutput=False)
    od = nc.declare_dram_parameter("out", [2, 128, N], F32, isOutput=True)

    with tile.TileContext(nc) as tc:
        with (
            tc.tile_pool(name="wts", bufs=1) as wts,
            tc.tile_pool(name="xs", bufs=1) as xs,
            tc.tile_pool(name="qk", bufs=1) as qkp,
            tc.tile_pool(name="vt", bufs=1) as vtp,
            tc.tile_pool(name="mega", bufs=3) as mega,
            tc.tile_pool(name="gph", bufs=4) as gph,
            tc.tile_pool(name="finp", bufs=2) as finp,
            tc.tile_pool(name="pse", bufs=EPS_BUFS, space="PSUM") as pse,
            tc.tile_pool(name="pso", bufs=1, space="PSUM") as pso,
            tc.tile_pool(name="psg", bufs=2, space="PSUM") as psg,
        ):
            def body(iv=None):
                # ---- input DMAs ----
                x_sb = [xs.tile([128, N], F32R, tag=f"x{i}", name=f"x_sb{i}")
                        for i in range(2)]
                for i in range(2):
                    for cch in range(4):
                        cs = slice(cch * (N // 4), (cch + 1) * (N // 4))
                        nc.sync.dma_start(x_sb[i][:, cs], xd[i][:, cs])
                wq_sb = wts.tile([128, 2, CQ], F32R, tag="wq", name="wq_sb")
                wk_sb = wts.tile([128, 2, CQ], F32R, tag="wk", name="wk_sb")
                wvq_sb = wts.tile([128, 2, CVQ], F32R, tag="wvq",
                                  name="wvq_sb")
                wg_sb = wts.tile([128, 2, C], BF16, tag="wg", name="wg_sb")
                for i in range(2):
                    nc.sync.dma_start(wq_sb[:, i, :], wqd[i])
                    nc.sync.dma_start(wk_sb[:, i, :], wkd[i])
                    nc.sync.dma_start(wvq_sb[:, i, :], wvqd[i])
                    nc.sync.dma_start(wg_sb[:, i, :], wgd[i])
                bq_sb = wts.tile([CQ, 1], F32, tag="bq", name="bq_sb")
                nc.sync.dma_start(bq_sb, bqd[:])
                bk_sb = wts.tile([CQ, 1], F32, tag="bk", name="bk_sb")
                nc.sync.dma_start(bk_sb, bkd[:])
                bvq_sb = wts.tile([1, CVQ], F32R, tag="bvq", name="bvq_sb")
                nc.sync.dma_start(bvq_sb, bvqd[:])
                bg_sb = wts.tile([128, 2], F32, tag="bg", name="bg_sb")
                for h in range(2):
                    nc.sync.dma_start(bg_sb[:, h:h + 1],
                                      bgd[h * 128:(h + 1) * 128, :])
                ones_row = wts.tile([1, 128], F32R, tag="ones_row",
                                    name="ones_row")
                nc.sync.dma_start(ones_row, onesd[:])
                ones_col = wts.tile([128, 1], BF16, tag="ones_col",
                                    name="ones_col")
                nc.vector.memset(ones_col, 1.0)

                q_sb = qkp.tile([2 * CQ, N], F32R, tag="q", name="q_sb")
                k_sb = qkp.tile([2 * CQ, N], F32R, tag="k", name="k_sb")
                vt_sb = vtp.tile([128, 32, C], BF16, tag="vt", name="vt_sb")
                qt_sb = vtp.tile([128, 32, CQ], BF16, tag="qt", name="qt_sb")
                pt_sb = wts.tile([CQ, C], F32R, tag="pt", name="pt_sb")
                sT_sb = wts.tile([1, C], F32, tag="sT", name="sT_sb")
                s_col = wts.tile([128, 2], BF16, tag="scol", name="s_col")
                bge_sb = wts.tile([128, 2], F32, tag="bge", name="bge_sb")

                # ---- q, k = conv1x1(x) + bias  [64, 4096] (f32) ----
                # interleaved with the v^T|q^T pass below for PE density
                def _scratch_ps(shape, name):
                    if GAMMA_ON_EPS:
                        return pse.tile(shape, F32, tag="eps", name=name)
                    return psg.tile(shape, F32, tag="gps", name=name)

                def emit_qk(ti):
                    sl = slice(ti * 512, (ti + 1) * 512)
                    for dst, w_s, b_s in ((q_sb, wq_sb, bq_sb),
                                          (k_sb, wk_sb, bk_sb)):
                        ps = _scratch_ps([CQ, 512], "qkps")
                        nc.tensor.matmul(ps, w_s[:, 0, :], x_sb[0][:, sl],
                                         start=True, stop=False)
                        nc.tensor.matmul(ps, w_s[:, 1, :], x_sb[1][:, sl],
                                         start=False, stop=True)
                        nc.scalar.activation(dst[:CQ, sl], ps, AF.Identity,
                                             bias=b_s, scale=1.0)

                # ---- v^T/N and q^T per 128-row chunk, fused [vt|qt] ----
                pt_ps = pse.tile([CQ, C], F32, tag="eps", name="pt_ps")

                def emit_vtq(ni):
                    nsl = slice(ni * 128, (ni + 1) * 128)
                    ps = _scratch_ps([128, CVQ], "vqps")
                    nc.tensor.matmul(ps, x_sb[0][:, nsl], wvq_sb[:, 0, :],
                                     start=True, stop=False)
                    nc.tensor.matmul(ps, x_sb[1][:, nsl], wvq_sb[:, 1, :],
                                     start=False, stop=False)
                    nc.tensor.matmul(ps, ones_row, bvq_sb,
                                     start=False, stop=True)
                    nc.vector.tensor_copy(vt_sb[:, ni, :], ps[:, 0:C])
                    nc.scalar.activation(qt_sb[:, ni, :], ps[:, C:CVQ],
                                         AF.Copy)
                    # P^T accumulation: P^T = q^T.T @ v^T = [64, 256]
                    nc.tensor.matmul(pt_ps, qt_sb[:, ni, :], vt_sb[:, ni, :],
                                     start=(ni == 0), stop=(ni == 31))

                for i in range(16):
                    if i < N // 512:
                        emit_qk(i)
                    emit_vtq(2 * i)
                    emit_vtq(2 * i + 1)
                # duplicate q/k into partitions 64..127 (PE row-group packing)
                for dst in (q_sb, k_sb):
                    nc.sync.dma_start(dst[CQ:2 * CQ, :], dst[:CQ, :])
                nc.scalar.activation(pt_sb, pt_ps, AF.Copy)

                # ---- S = rowsum(v/N); bg_eff = bg - Wg S ----
                sps = pso.tile([1, C], F32, tag="ob", name="sps")
                for ni in range(32):
                    nc.tensor.matmul(sps, ones_col, vt_sb[:, ni, :],
                                     start=(ni == 0), stop=(ni == 31))
                nc.vector.tensor_copy(sT_sb, sps)
                for h in range(2):
                    # [1,128] row -> [128,1] column (with f32->bf16 cast)
                    nc.gpsimd.dma_start(s_col[:, h:h + 1],
                                        sT_sb[:, h * 128:(h + 1) * 128])
                for h in range(2):
                    hsl = slice(h * 128, (h + 1) * 128)
                    ps = pso.tile([128, 1], F32, tag="ob", name="bgps")
                    nc.tensor.matmul(ps, wg_sb[:, 0, hsl], s_col[:, 0:1],
                                     start=True, stop=False)
                    nc.tensor.matmul(ps, wg_sb[:, 1, hsl], s_col[:, 1:2],
                                     start=False, stop=True)
                    nc.scalar.activation(bge_sb[:, h:h + 1], ps, AF.Identity,
                                         bias=bg_sb[:, h:h + 1], scale=-1.0)

                # ---- main attention loop (software-pipelined) ----
                pairs = [(mt, p) for mt in range(N // MT) for p in range(NP)]
                eps_t = {}
                rm_t = {}
                tm_t = {}
                g_t = {}
                o_ps = {}

                def emit_E(j):
                    mt, p = pairs[j]
                    msl = slice(mt * MT, (mt + 1) * MT)
                    nA, nB = 2 * p, 2 * p + 1
                    eps = pse.tile([128, 2 * MT], F32, tag="eps", name="eps")
                    nc.tensor.matmul(eps[:, 0:MT],
                                     q_sb[:CQ, nA * 128:(nA + 1) * 128],
                                     k_sb[:CQ, msl], start=True, stop=True)
                    nc.tensor.matmul(eps[:, MT:2 * MT],
                                     q_sb[CQ:2 * CQ, nB * 128:(nB + 1) * 128],
                                     k_sb[CQ:2 * CQ, msl],
                                     start=True, stop=True)
                    eps_t[j] = eps

                def emit_r(j):
                    g = j // EGRP
                    q_ = j % EGRP
                    if q_ == 0:
                        rm_t[g] = mega.tile([128, EGRP * 2 * MT], BF16,
                                            tag="rm", name="rm")
                    eps = eps_t.pop(j)
                    dst = rm_t[g][:, q_ * 2 * MT:(q_ + 1) * 2 * MT]
                    if ACT_R_EVERY and j % ACT_R_EVERY == ACT_R_EVERY - 1:
                        nc.scalar.activation(dst, eps, AF.Relu, scale=-1.0)
                    else:
                        nc.vector.tensor_scalar(dst, eps, -1.0, 0.0,
                                                AL.mult, AL.max)

                def emit_exp(g):
                    tm_t[g] = mega.tile([128, EGRP * 2 * MT], BF16,
                                        tag="tm", name="tm")
                    nc.scalar.activation(tm_t[g], rm_t[g], AF.Exp, scale=-1.0)

                def emit_combine(j):
                    g = j // EGRP
                    q_ = j % EGRP
                    sl = slice(q_ * 2 * MT, (q_ + 1) * 2 * MT)
                    g16 = gph.tile([128, 2 * MT], BF16, tag="g", name="g16")
                    if POOL_COMB_EVERY and j % POOL_COMB_EVERY == (
                            POOL_COMB_EVERY - 1):
                        nc.gpsimd.tensor_tensor(g16, tm_t[g][:, sl],
                                                rm_t[g][:, sl], AL.add)
                    else:
                        nc.vector.tensor_tensor(g16, tm_t[g][:, sl],
                                                rm_t[g][:, sl], AL.add)
                    g_t[j] = g16
                    if q_ == EGRP - 1:
                        del rm_t[g], tm_t[g]

                def emit_out(j):
                    mt, p = pairs[j]
                    nA, nB = 2 * p, 2 * p + 1
                    g16 = g_t.pop(j)
                    if p == 0:
                        o_ps[mt] = [pso.tile([128, MT], F32,
                                             tag=("oa", "ob")[ci],
                                             name=f"o_ps{ci}")
                                    for ci in range(2)]
                    for ci in range(2):
                        csl = slice(ci * 128, (ci + 1) * 128)
                        nc.tensor.matmul(o_ps[mt][ci], vt_sb[:, nA, csl],
                                         g16[:, 0:MT], start=(p == 0),
                                         stop=False)
                        nc.tensor.matmul(o_ps[mt][ci], vt_sb[:, nB, csl],
                                         g16[:, MT:2 * MT], start=False,
                                         stop=False)

                def emit_corr(mt):
                    # E-term: o_ps[ci] += P^T[:, ci-chunk].T @ k[:, msl]
                    msl = slice(mt * MT, (mt + 1) * MT)
                    for ci in range(2):
                        csl = slice(ci * 128, (ci + 1) * 128)
                        nc.tensor.matmul(o_ps[mt][ci], pt_sb[:, csl],
                                         k_sb[:CQ, msl], start=False,
                                         stop=True)

                def emit_gamma(mt):
                    msl = slice(mt * MT, (mt + 1) * MT)
                    osb = []
                    for ci in range(2):
                        ob = finp.tile([128, MT], BF16, tag=f"ob{ci}",
                                       name=f"ob{ci}")
                        if ci == 0:
                            nc.scalar.activation(ob, o_ps[mt][ci], AF.Copy)
                        else:
                            nc.vector.tensor_copy(ob, o_ps[mt][ci])
                        osb.append(ob)
                    del o_ps[mt]
                    for h in range(2):
                        hsl = slice(h * 128, (h + 1) * 128)
                        gps = _scratch_ps([128, MT], "gps")
                        nc.tensor.matmul(gps, wg_sb[:, 0, hsl], osb[0],
                                         start=True, stop=False)
                        nc.tensor.matmul(gps, wg_sb[:, 1, hsl], osb[1],
                                         start=False, stop=True)
                        fo = finp.tile([128, MT], F32, tag="fo", name="fo")
                        nc.scalar.activation(fo, gps, AF.Identity,
                                             bias=bge_sb[:, h:h + 1],
                                             scale=1.0)
                        nc.sync.dma_start(od[h, :, msl], fo)

                for s in range(NPAIR + LAG_O):
                    if s < NPAIR:
                        emit_E(s)
                        emit_r(s)
                        if s % EGRP == EGRP - 1:
                            emit_exp(s // EGRP)
                    c = s - LAG_C
                    if 0 <= c < NPAIR:
                        emit_combine(c)
                    o = s - LAG_O
                    if 0 <= o < NPAIR:
                        emit_out(o)
                        if pairs[o][1] == NP - 1:
                            emit_corr(pairs[o][0])
                            emit_gamma(pairs[o][0])

            if reps == 1:
                body()
            else:
                with tc.For_i(0, reps, 1):
                    body()
    nc.compile()
    return nc


_NC_CACHE = {}


def _get_nc(reps=1, variant=None):
    key = (reps, EGRP, LAG_C, LAG_O, ACT_R_EVERY, POOL_COMB_EVERY,
           EPS_BUFS, GAMMA_ON_EPS)
    if key not in _NC_CACHE:
        _NC_CACHE[key] = build_nc(reps)
    return _NC_CACHE[key]


def _prep_in_maps(inputs):
    x = np.ascontiguousarray(np.asarray(inputs["x"], dtype=np.float32))
    wq = np.asarray(inputs["query_weight"], np.float32)[:, :, 0, 0]
    bq = np.asarray(inputs["query_bias"], np.float32)
    wk = np.asarray(inputs["key_weight"], np.float32)[:, :, 0, 0]
    bk = np.asarray(inputs["key_bias"], np.float32)
    wv = np.asarray(inputs["value_weight"], np.float32)[:, :, 0, 0]
    bv = np.asarray(inputs["value_bias"], np.float32)
    wg = np.asarray(inputs["gamma_weight"], np.float32)[:, :, 0, 0]
    bg = np.asarray(inputs["gamma_bias"], np.float32)

    wqT = np.ascontiguousarray(wq.T).reshape(2, 128, CQ)
    wkT = np.ascontiguousarray(wk.T).reshape(2, 128, CQ)
    wvqT = np.ascontiguousarray(
        np.concatenate([wv.T / N, wq.T], axis=1)).reshape(2, 128, CVQ)
    bvq = np.concatenate([bv / N, bq]).reshape(1, CVQ)
    wgT16 = np.ascontiguousarray(wg.T).astype(ml_dtypes.bfloat16).reshape(
        2, 128, C)
    shared = {
        "wqT": wqT, "wkT": wkT,
        "bq": np.ascontiguousarray(bq.reshape(CQ, 1)),
        "bk": np.ascontiguousarray(bk.reshape(CQ, 1)),
        "wvqT": wvqT, "bvq": bvq, "wgT16": wgT16,
        "bg": np.ascontiguousarray(bg.reshape(C, 1)),
        "ones": np.ones((1, 128), np.float32),
    }
    return [dict(shared, x=x[b].reshape(2, 128, N))
            for b in range(B)]


def _run(inputs, trace=False, reps=1, variant=None):
    nc = _get_nc(reps)
    in_maps = _prep_in_maps(inputs)
    res = run_bass_kernel_spmd(nc, in_maps, core_ids=list(range(NCORES)),
                               trace=trace)
    out = np.stack([r["out"].reshape(C, HH, WW) for r in res.results], axis=0)
    return out, res


def kernel(**inputs):
    out, _ = _run(inputs, trace=False)
    return out
